# revision 1
# baseline (speedup 1.0000x reference)
"""Trainium2 Bass kernel for 2-layer GAT (nn_GAT_33337536151585).

Strategy (8 NeuronCores, SPMD):
  - Nodes padded to 50176 = 8*49*128, partitioned contiguously: core c owns
    nodes [c*6272, (c+1)*6272) = 49 tiles of 128 dst nodes.
  - Edges sorted by dst, assigned to the dst-owning core. Per dst tile the
    edges are packed into chunks of 128 slots (slot i -> partition i%128,
    chunk i//128) split into a low-src group and high-src group so gathers
    can use int16 indices (dma_gather limit): chunk counts CHL/CHH uniform
    across all cores/tiles (SPMD); pad slots gather row 0 and have
    dst_local = -1 so the one-hot kills their contribution.
  - Phase A (replicated on every core): rec1[n] = [x@W1 | x@V1l | x@V1r |pad]
    (1280B rows) where V1 = W1 @ blockdiag(al1/ar1); x^T streamed from the
    host so no on-chip transpose. Also computes per-core own-node er table
    my_er1[6272, 64] (=[el|er|pad], 256B rows) from a per-core xT_own input.
  - Phase B (layer 1) per owned dst tile: dma_gather rec1 rows by src
    (low/high passes), dma_gather my_er1 rows by LOCAL dst,
    scores=leaky_relu(el_src+er_dst), ex=exp(scores) (no segment max needed:
    |score| <= ~10), one-hot S[e,v]=(dst_local==v) via DVE compare with an
    iota matrix, PE matmul psum[v,0:260] += S^T @ [ex*feat | ex] accumulates
    numerator and softmax denominator at once; out=num/den + b1, ELU;
    rec2 = [h1@W2 | el2 | er2 | pad] (256B rows) via PE-transpose + matmul
    -> my_rec2[6272, 64].
  - AllGather my_rec2 (12.8MB total) + copy Shared->Local (indirect reads
    from the Shared address space are not supported).
  - Phase C (layer 2): same machinery, 1 head, 18-wide payload, then
    log_softmax.
  - All matmul operands are float32r (full-rate fp32 PE mode, ~1e-4 rounding
    that is well inside tolerance).
All float math runs on-device; the host only reorders indices/layouts.
"""

import numpy as np

# problem constants
N = 50000
E = 800000
NFEAT = 256
NHID = 64
HEADS = 4
NCLASS = 16
NEG = 0.2
NCORES = 8
P = 128

F1 = HEADS * NHID          # 256
REC1W = 320                # feat(256) | el(4) | er(4) | pad -> 1280B rows
RECSW = 64                 # small-table row: 256B
TILES_PC = 49
NPC = TILES_PC * P         # 6272
NPAD = NCORES * NPC        # 50176


def preprocess(inputs, ncores=NCORES, tiles_pc=TILES_PC):
    """Host-side index/layout preprocessing."""
    x = np.asarray(inputs["x"], np.float32)
    src = np.asarray(inputs["src"], np.int64)
    dst = np.asarray(inputs["dst"], np.int64)
    W1 = np.asarray(inputs["W1"], np.float32)
    al1 = np.asarray(inputs["al1"], np.float32)
    ar1 = np.asarray(inputs["ar1"], np.float32)
    b1 = np.asarray(inputs["b1"], np.float32)
    W2 = np.asarray(inputs["W2"], np.float32)
    al2 = np.asarray(inputs["al2"], np.float32)
    ar2 = np.asarray(inputs["ar2"], np.float32)
    b2 = np.asarray(inputs["b2"], np.float32)

    n_nodes = x.shape[0]
    nf = x.shape[1]
    npc = tiles_pc * P
    npad = ncores * npc
    split = npad // 2
    assert split % P == 0 and split < 32768 and npad >= n_nodes

    x_pad = np.zeros((npad, nf), np.float32)
    x_pad[:n_nodes] = x
    xT = np.ascontiguousarray(x_pad.T)  # [nf, npad]

    def fused_rhs(W, al, ar):
        heads, dh = al.shape
        fout = W.shape[1]
        AlAr = np.zeros((fout, 2 * heads), np.float64)
        for h in range(heads):
            AlAr[h * dh:(h + 1) * dh, h] = al[h]
            AlAr[h * dh:(h + 1) * dh, heads + h] = ar[h]
        V = (W.astype(np.float64) @ AlAr).astype(np.float32)
        return np.ascontiguousarray(np.concatenate([W, V], axis=1))

    rhs1 = fused_rhs(W1, al1, ar1)            # [nf, 264]
    rhs2 = fused_rhs(W2, al2, ar2)            # [256, 18]

    # edge partitioning
    order = np.argsort(dst, kind="stable")
    dsts = dst[order].astype(np.int64)
    srcs = src[order].astype(np.int64)
    ntiles = npad // P
    tile_of = dsts // P
    counts = np.bincount(tile_of, minlength=ntiles)
    starts = np.zeros(ntiles + 1, np.int64)
    np.cumsum(counts, out=starts[1:])

    # per-tile low/high group sizes
    lo_cnt = np.zeros(ntiles, np.int64)
    for t in range(ntiles):
        s = srcs[starts[t]:starts[t + 1]]
        lo_cnt[t] = int((s < split).sum())
    hi_cnt = counts - lo_cnt
    chl = max(1, int(np.max((lo_cnt + P - 1) // P)))
    chh = max(1, int(np.max((hi_cnt + P - 1) // P)))
    ch = chl + chh

    def wrap16(idx_flat, cols):
        # dma_gather index layout: index i -> [i%16, i//16], and the 16-row
        # block replicated across all 8 gpsimd Q7 cores (128 partitions)
        out = np.zeros((16, cols), np.int16)
        n = len(idx_flat)
        out[np.arange(n) % 16, np.arange(n) // 16] = idx_flat.astype(np.int16)
        return np.tile(out, (P // 16, 1))

    idx_cols = (chl + chh + ch) * 8
    eidx, edl_l = [], []
    for c in range(ncores):
        ecols = np.zeros((tiles_pc, P, idx_cols), np.int16)
        edl = np.full((tiles_pc, P, ch), -1.0, np.float32)
        for tl in range(tiles_pc):
            t = c * tiles_pc + tl
            s0, s1 = starts[t], starts[t + 1]
            es = srcs[s0:s1]
            ed = dsts[s0:s1]
            lo_mask = es < split
            es_lo, ed_lo = es[lo_mask], ed[lo_mask]
            es_hi, ed_hi = es[~lo_mask], ed[~lo_mask]
            nlo, nhi = len(es_lo), len(es_hi)
            ilo = np.zeros(chl * P, np.int64)
            ilo[:nlo] = es_lo
            ihi = np.zeros(chh * P, np.int64)
            ihi[:nhi] = es_hi - split
            dl = np.zeros(ch * P, np.int64)
            dl[:nlo] = ed_lo - c * npc
            dl[chl * P:chl * P + nhi] = ed_hi - c * npc
            ecols[tl, :, 0:chl * 8] = wrap16(ilo, chl * 8)
            ecols[tl, :, chl * 8:(chl + chh) * 8] = wrap16(ihi, chh * 8)
            ecols[tl, :, (chl + chh) * 8:] = wrap16(dl, ch * 8)
            pos_lo = np.arange(nlo)
            edl[tl, pos_lo % P, pos_lo // P] = (ed_lo - t * P).astype(np.float32)
            pos_hi = chl * P + np.arange(nhi)
            edl[tl, pos_hi % P, pos_hi // P] = (ed_hi - t * P).astype(np.float32)
        eidx.append(ecols)
        edl_l.append(edl)

    xT_own = [np.ascontiguousarray(xT[:, c * npc:(c + 1) * npc])
              for c in range(ncores)]

    consts = dict(
        xT=xT,
        rhs1=rhs1,
        rhs2=rhs2,
        b1_bc=np.ascontiguousarray(np.broadcast_to(b1, (P, b1.shape[0]))).astype(np.float32),
        b2_bc=np.ascontiguousarray(np.broadcast_to(b2, (P, b2.shape[0]))).astype(np.float32),
        J=np.ascontiguousarray(np.broadcast_to(np.arange(P, dtype=np.float32), (P, P))),
    )
    return dict(consts=consts, eidx=eidx, edl=edl_l, xT_own=xT_own,
                chl=chl, chh=chh, ch=ch, npad=npad, npc=npc, split=split,
                tiles_pc=tiles_pc, ncores=ncores, nf=nf)


def build_nc(chl, chh, ncores=NCORES, tiles_pc=TILES_PC, nf=NFEAT,
             linearize=False, dbg=False, reps=1, rep_phases='ABC'):
    """Build + compile the SPMD Bass program."""
    import concourse.bass as bass
    import concourse.bacc as bacc
    import concourse.tile as tile
    from concourse import mybir
    from concourse.masks import make_identity

    f32 = mybir.dt.float32
    f32r = mybir.dt.float32r
    i16 = mybir.dt.int16
    AF = mybir.ActivationFunctionType
    OP = mybir.AluOpType

    ch = chl + chh
    npc = tiles_pc * P
    npad = ncores * npc
    split = npad // 2
    ntiles = npad // P
    heads = HEADS
    dh = NHID
    f1 = heads * dh
    r1w = REC1W
    rsw = RECSW
    ncls = NCLASS
    m2w = ncls + 2

    nc = bacc.Bacc("TRN2", target_bir_lowering=False, debug=False,
                   num_devices=ncores)

    # I/O
    xT_d = nc.dram_tensor("xT", [nf, npad], f32r, kind="ExternalInput")
    xTo_d = nc.dram_tensor("xT_own", [nf, npc], f32r, kind="ExternalInput")
    rhs1_d = nc.dram_tensor("rhs1", [nf, f1 + 2 * heads], f32r, kind="ExternalInput")
    rhs2_d = nc.dram_tensor("rhs2", [f1, m2w], f32r, kind="ExternalInput")
    b1_d = nc.dram_tensor("b1_bc", [P, f1], f32, kind="ExternalInput")
    b2_d = nc.dram_tensor("b2_bc", [P, ncls], f32, kind="ExternalInput")
    J_d = nc.dram_tensor("J", [P, P], f32, kind="ExternalInput")
    eidx_d = nc.dram_tensor("eidx", [tiles_pc, P, (chl + chh + ch) * 8], i16,
                            kind="ExternalInput")
    edl_d = nc.dram_tensor("edl", [tiles_pc, P, ch], f32, kind="ExternalInput")
    y_d = nc.dram_tensor("y", [npc, ncls], f32, kind="ExternalOutput")
    # internal DRAM
    io_dbg = dict(kind="ExternalOutput") if dbg else {}
    rec1 = nc.dram_tensor("rec1", [npad, r1w], f32, **io_dbg)
    my_er1 = nc.dram_tensor("my_er1", [npc, rsw], f32)
    my_rec2 = nc.dram_tensor("my_rec2", [npc, rsw], f32)
    all_rec2_sh = nc.dram_tensor("all_rec2_sh", [npad, rsw], f32, addr_space="Shared")
    all_rec2 = nc.dram_tensor("all_rec2", [npad, rsw], f32, **io_dbg)

    rw = f1 + 2 * heads  # 264, rhs1 width
    mw = f1 + heads      # 260, msg_ex width

    with tile.TileContext(nc, linearize=linearize) as tc:
        with tc.tile_pool(name="consts", bufs=1) as cpool:
            nk1 = nf // P
            rhs1_sb = [cpool.tile([P, rw], f32r, tag=f"rhs1_{k}", name=f"rhs1_sb{k}")
                       for k in range(nk1)]
            for k in range(nk1):
                nc.sync.dma_start(out=rhs1_sb[k][:], in_=rhs1_d[k * P:(k + 1) * P, :])
            nk2 = f1 // P
            rhs2_sb = [cpool.tile([P, m2w], f32r, tag=f"rhs2_{k}", name=f"rhs2_sb{k}")
                       for k in range(nk2)]
            for k in range(nk2):
                nc.sync.dma_start(out=rhs2_sb[k][:], in_=rhs2_d[k * P:(k + 1) * P, :])
            b1t = cpool.tile([P, f1], f32, tag="b1t")
            nc.sync.dma_start(out=b1t[:], in_=b1_d[:, :])
            b2t = cpool.tile([P, ncls], f32, tag="b2t")
            nc.sync.dma_start(out=b2t[:], in_=b2_d[:, :])
            Jt = cpool.tile([P, P], f32, tag="Jt")
            nc.sync.dma_start(out=Jt[:], in_=J_d[:, :])
            ident = cpool.tile([P, P], f32, tag="ident")
            make_identity(nc, ident[:])

            # ---------------- Phase A: rec1 for ALL nodes (replicated)
            ra_ = reps if 'A' in rep_phases else 1
            rb_ = reps if 'B' in rep_phases else 1
            rc_ = reps if 'C' in rep_phases else 1
            for _rep in range(1):
                for _ra in range(ra_):
                  with (tc.tile_pool(name="pA", bufs=3) as pA,
                      tc.tile_pool(name="psA", bufs=2, space="PSUM") as psA):
                    for t in range(ntiles):
                        xts = []
                        for k in range(nk1):
                            xt = pA.tile([P, P], f32r, tag=f"xt{k}", name=f"xt{k}")
                            nc.sync.dma_start(
                                out=xt[:],
                                in_=xT_d[k * P:(k + 1) * P, t * P:(t + 1) * P])
                            xts.append(xt)
                        ps = psA.tile([P, rw], f32, tag="psA")
                        for k in range(nk1):
                            nc.tensor.matmul(ps[:], lhsT=xts[k][:], rhs=rhs1_sb[k][:],
                                             start=(k == 0), stop=(k == nk1 - 1))
                        ra = pA.tile([P, rw], f32, tag="ra")
                        if t % 2 == 0:
                            nc.vector.tensor_copy(ra[:], ps[:])
                        else:
                            nc.scalar.copy(ra[:], ps[:])
                        nc.sync.dma_start(
                            out=rec1[:].rearrange("(t p) w -> t p w", p=P)[t, :, 0:rw],
                            in_=ra[:])
                    # own-node el/er table
                    for tl in range(tiles_pc):
                        xos = []
                        for k in range(nk1):
                            xo = pA.tile([P, P], f32r, tag=f"xt{k}", name=f"xo{k}")
                            nc.sync.dma_start(
                                out=xo[:],
                                in_=xTo_d[k * P:(k + 1) * P, tl * P:(tl + 1) * P])
                            xos.append(xo)
                        pse = psA.tile([P, 8], f32, tag="psE")
                        for k in range(nk1):
                            nc.tensor.matmul(pse[:], lhsT=xos[k][:],
                                             rhs=rhs1_sb[k][:, f1:f1 + 8],
                                             start=(k == 0), stop=(k == nk1 - 1))
                        re = pA.tile([P, 8], f32, tag="re")
                        nc.vector.tensor_copy(re[:], pse[:])
                        nc.sync.dma_start(
                            out=my_er1[:].rearrange("(t p) w -> t p w", p=P)[tl, :, 0:8],
                            in_=re[:])

                tc.strict_bb_all_engine_barrier()

                def gather_edges(pool, tl, table_lo, table_hi, table_er, elem, tag):
                    """Per-tile gathers: rec rows by src (low/high) + er rows by
                    local dst."""
                    it = pool.tile([P, (chl + chh + ch) * 8], i16, tag=f"it{tag}",
                                   name=f"it{tag}")
                    nc.sync.dma_start(out=it[:], in_=eidx_d[tl, :, :])
                    rec_g = pool.tile([P, ch * elem], f32, tag=f"rg{tag}",
                                      name=f"rg{tag}")
                    rg3 = rec_g[:].rearrange("p (c w) -> p c w", w=elem)
                    nc.gpsimd.dma_gather(
                        out_ap=rg3[:, 0:chl, :], in_ap=table_lo,
                        idxs_ap=it[:, 0:chl * 8], num_idxs=chl * P,
                        num_idxs_reg=chl * P, elem_size=elem, single_packet=False)
                    nc.gpsimd.dma_gather(
                        out_ap=rg3[:, chl:ch, :], in_ap=table_hi,
                        idxs_ap=it[:, chl * 8:(chl + chh) * 8], num_idxs=chh * P,
                        num_idxs_reg=chh * P, elem_size=elem, single_packet=False)
                    er_g = pool.tile([P, ch * rsw], f32, tag=f"eg{tag}",
                                     name=f"eg{tag}")
                    nc.gpsimd.dma_gather(
                        out_ap=er_g[:].rearrange("p (c w) -> p c w", w=rsw),
                        in_ap=table_er,
                        idxs_ap=it[:, (chl + chh) * 8:(chl + chh + ch) * 8],
                        num_idxs=ch * P, num_idxs_reg=ch * P, elem_size=rsw,
                        single_packet=False)
                    return rec_g, er_g

                # ---------------- Phase B: layer-1 aggregation for owned tiles
                for _rb in range(rb_):
                  with (tc.tile_pool(name="pB", bufs=2) as pB,
                      tc.tile_pool(name="psB", bufs=2, space="PSUM") as psB,
                      tc.tile_pool(name="psT", bufs=2, space="PSUM") as psT):
                    for tl in range(tiles_pc):
                        rec_g, er_g = gather_edges(
                            pB, tl, rec1[0:split, :], rec1[split:npad, :],
                            my_er1[:], r1w, "B")
                        edl = pB.tile([P, ch], f32, tag="edl")
                        nc.sync.dma_start(out=edl[:], in_=edl_d[tl, :, :])

                        rg3 = rec_g[:].rearrange("p (c w) -> p c w", w=r1w)
                        eg3 = er_g[:].rearrange("p (c w) -> p c w", w=rsw)
                        sc = pB.tile([P, ch * heads], f32, tag="sc")
                        sc3 = sc[:].rearrange("p (c w) -> p c w", w=heads)
                        nc.vector.tensor_tensor(
                            out=sc3, in0=rg3[:, :, f1:f1 + heads],
                            in1=eg3[:, :, 4:8], op=OP.add)
                        neg = pB.tile([P, ch * heads], f32, tag="neg")
                        nc.vector.tensor_scalar(
                            out=neg[:], in0=sc[:], scalar1=0.0, scalar2=NEG,
                            op0=OP.min, op1=OP.mult)
                        pos = pB.tile([P, ch * heads], f32, tag="pos")
                        nc.vector.tensor_scalar(
                            out=pos[:], in0=sc[:], scalar1=0.0, scalar2=None,
                            op0=OP.max)
                        lk = pB.tile([P, ch * heads], f32, tag="lk")
                        nc.vector.tensor_tensor(out=lk[:], in0=neg[:], in1=pos[:],
                                                op=OP.add)
                        msg_ex = pB.tile([P, ch * mw], f32r, tag="msg_ex")
                        me3 = msg_ex[:].rearrange("p (c w) -> p c w", w=mw)
                        nc.scalar.activation(
                            out=me3[:, :, f1:f1 + heads],
                            in_=lk[:].rearrange("p (c w) -> p c w", w=heads),
                            func=AF.Exp)
                        me4 = me3[:, :, 0:f1].rearrange("p c (h d) -> p c h d", d=dh)
                        nc.vector.tensor_tensor(
                            out=me4,
                            in0=rg3[:, :, 0:f1].rearrange("p c (h d) -> p c h d", d=dh),
                            in1=me3[:, :, f1:f1 + heads].to_broadcast([P, ch, heads, dh]),
                            op=OP.mult)
                        S = pB.tile([P, ch * P], f32r, tag="S")
                        nc.vector.tensor_tensor(
                            out=S[:].rearrange("p (c v) -> p c v", v=P),
                            in0=edl[:].to_broadcast([P, ch, P]),
                            in1=Jt[:].unsqueeze(1).to_broadcast([P, ch, P]),
                            op=OP.is_equal)
                        psU = psB.tile([P, mw], f32, tag="psU")
                        for j in range(ch):
                            nc.tensor.matmul(
                                psU[:],
                                lhsT=S[:, j * P:(j + 1) * P],
                                rhs=msg_ex[:, j * mw:(j + 1) * mw],
                                start=(j == 0), stop=(j == ch - 1))
                        den = pB.tile([P, heads], f32, tag="den")
                        nc.vector.tensor_scalar(
                            out=den[:], in0=psU[:, f1:f1 + heads], scalar1=1e-30,
                            scalar2=None, op0=OP.max)
                        denr = pB.tile([P, heads], f32, tag="denr")
                        nc.vector.reciprocal(denr[:], den[:])
                        h1 = pB.tile([P, f1], f32, tag="h1")
                        nc.vector.tensor_tensor(
                            out=h1[:].rearrange("p (h d) -> p h d", d=dh),
                            in0=psU[:, 0:f1].rearrange("p (h d) -> p h d", d=dh),
                            in1=denr[:].to_broadcast([P, heads, dh]),
                            op=OP.mult)
                        hb = pB.tile([P, f1], f32, tag="hb")
                        nc.vector.tensor_tensor(out=hb[:], in0=h1[:], in1=b1t[:],
                                                op=OP.add)
                        # ELU = relu(x) + exp(min(x,0)) - 1
                        zm = pB.tile([P, f1], f32, tag="zm")
                        nc.vector.tensor_scalar(
                            out=zm[:], in0=hb[:], scalar1=0.0, scalar2=None,
                            op0=OP.min)
                        ez = pB.tile([P, f1], f32, tag="ez")
                        nc.scalar.activation(out=ez[:], in_=zm[:], func=AF.Exp)
                        rp = pB.tile([P, f1], f32, tag="rp")
                        nc.scalar.activation(out=rp[:], in_=hb[:], func=AF.Relu)
                        h1s = pB.tile([P, f1], f32, tag="h1s")
                        nc.vector.tensor_tensor(out=h1s[:], in0=ez[:], in1=rp[:],
                                                op=OP.add)
                        h1f = pB.tile([P, f1], f32, tag="h1f")
                        nc.vector.tensor_scalar(
                            out=h1f[:], in0=h1s[:], scalar1=-1.0, scalar2=None,
                            op0=OP.add)
                        # rec2 = [h1f @ W2 | el2 | er2] via PE transpose
                        ps2 = psT.tile([P, m2w], f32, tag="ps2")
                        for k in range(nk2):
                            pst = psT.tile([P, P], f32, tag="pst")
                            nc.tensor.transpose(pst[:], h1f[:, k * P:(k + 1) * P],
                                                ident[:])
                            hT = pB.tile([P, P], f32r, tag="hT")
                            if k % 2 == 0:
                                nc.vector.tensor_copy(hT[:], pst[:])
                            else:
                                nc.scalar.copy(hT[:], pst[:])
                            nc.tensor.matmul(ps2[:], lhsT=hT[:], rhs=rhs2_sb[k][:],
                                             start=(k == 0), stop=(k == nk2 - 1))
                        r2 = pB.tile([P, m2w], f32, tag="r2")
                        nc.vector.tensor_copy(r2[:], ps2[:])
                        nc.sync.dma_start(
                            out=my_rec2[:].rearrange("(t p) w -> t p w", p=P)[tl, :, 0:m2w],
                            in_=r2[:])

                tc.strict_bb_all_engine_barrier()
                nc.gpsimd.collective_compute(
                    "AllGather", mybir.AluOpType.bypass,
                    replica_groups=[list(range(ncores))],
                    ins=[my_rec2.ap()], outs=[all_rec2_sh.ap()])
                tc.strict_bb_all_engine_barrier()
                nc.sync.dma_start(out=all_rec2[:, :], in_=all_rec2_sh[:, :])
                tc.strict_bb_all_engine_barrier()

                # ---------------- Phase C: layer-2 aggregation + log_softmax
                for _rc in range(rc_):
                  with (tc.tile_pool(name="pC", bufs=2) as pC,
                      tc.tile_pool(name="psC", bufs=2, space="PSUM") as psC):
                    for tl in range(tiles_pc):
                        rec_g, er_g = gather_edges(
                            pC, tl, all_rec2[0:split, :], all_rec2[split:npad, :],
                            my_rec2[:], rsw, "C")
                        edl = pC.tile([P, ch], f32, tag="edl2")
                        nc.sync.dma_start(out=edl[:], in_=edl_d[tl, :, :])

                        rg3 = rec_g[:].rearrange("p (c w) -> p c w", w=rsw)
                        eg3 = er_g[:].rearrange("p (c w) -> p c w", w=rsw)
                        sc = pC.tile([P, ch], f32, tag="sc2")
                        nc.vector.tensor_tensor(
                            out=sc[:].unsqueeze(2),
                            in0=rg3[:, :, ncls:ncls + 1],
                            in1=eg3[:, :, ncls + 1:ncls + 2], op=OP.add)
                        neg = pC.tile([P, ch], f32, tag="neg2")
                        nc.vector.tensor_scalar(
                            out=neg[:], in0=sc[:], scalar1=0.0, scalar2=NEG,
                            op0=OP.min, op1=OP.mult)
                        pos = pC.tile([P, ch], f32, tag="pos2")
                        nc.vector.tensor_scalar(
                            out=pos[:], in0=sc[:], scalar1=0.0, scalar2=None,
                            op0=OP.max)
                        lk = pC.tile([P, ch], f32, tag="lk2")
                        nc.vector.tensor_tensor(out=lk[:], in0=neg[:], in1=pos[:],
                                                op=OP.add)
                        m2ex = pC.tile([P, ch * m2w], f32r, tag="m2ex")
                        m23 = m2ex[:].rearrange("p (c w) -> p c w", w=m2w)
                        nc.scalar.activation(out=m23[:, :, ncls:ncls + 1],
                                             in_=lk[:].unsqueeze(2), func=AF.Exp)
                        nc.scalar.activation(out=m23[:, :, ncls + 1:ncls + 2],
                                             in_=lk[:].unsqueeze(2), func=AF.Exp)
                        nc.vector.tensor_tensor(
                            out=m23[:, :, 0:ncls],
                            in0=rg3[:, :, 0:ncls],
                            in1=m23[:, :, ncls:ncls + 1].to_broadcast([P, ch, ncls]),
                            op=OP.mult)
                        S = pC.tile([P, ch * P], f32r, tag="S2")
                        nc.vector.tensor_tensor(
                            out=S[:].rearrange("p (c v) -> p c v", v=P),
                            in0=edl[:].to_broadcast([P, ch, P]),
                            in1=Jt[:].unsqueeze(1).to_broadcast([P, ch, P]),
                            op=OP.is_equal)
                        psU = psC.tile([P, m2w], f32, tag="psU2")
                        for j in range(ch):
                            nc.tensor.matmul(
                                psU[:],
                                lhsT=S[:, j * P:(j + 1) * P],
                                rhs=m2ex[:, j * m2w:(j + 1) * m2w],
                                start=(j == 0), stop=(j == ch - 1))
                        den = pC.tile([P, 1], f32, tag="den2")
                        nc.vector.tensor_scalar(
                            out=den[:], in0=psU[:, ncls:ncls + 1], scalar1=1e-30,
                            scalar2=None, op0=OP.max)
                        denr = pC.tile([P, 1], f32, tag="denr2")
                        nc.vector.reciprocal(denr[:], den[:])
                        lg0 = pC.tile([P, ncls], f32, tag="lg0")
                        nc.vector.tensor_tensor(
                            out=lg0[:], in0=psU[:, 0:ncls],
                            in1=denr[:].to_broadcast([P, ncls]), op=OP.mult)
                        lg = pC.tile([P, ncls], f32, tag="lg")
                        nc.vector.tensor_tensor(out=lg[:], in0=lg0[:], in1=b2t[:],
                                                op=OP.add)
                        mx = pC.tile([P, 1], f32, tag="mx")
                        nc.vector.tensor_reduce(out=mx[:], in_=lg[:],
                                                axis=mybir.AxisListType.X, op=OP.max)
                        sh = pC.tile([P, ncls], f32, tag="sh")
                        nc.vector.tensor_tensor(
                            out=sh[:], in0=lg[:], in1=mx[:].to_broadcast([P, ncls]),
                            op=OP.subtract)
                        es = pC.tile([P, ncls], f32, tag="es")
                        sm = pC.tile([P, 1], f32, tag="sm")
                        nc.scalar.activation(out=es[:], in_=sh[:], func=AF.Exp,
                                             accum_out=sm[:])
                        lns = pC.tile([P, 1], f32, tag="lns")
                        nc.scalar.activation(out=lns[:], in_=sm[:], func=AF.Ln)
                        yt = pC.tile([P, ncls], f32, tag="yt")
                        nc.vector.tensor_tensor(
                            out=yt[:], in0=sh[:], in1=lns[:].to_broadcast([P, ncls]),
                            op=OP.subtract)
                        nc.sync.dma_start(out=y_d[tl * P:(tl + 1) * P, :], in_=yt[:])

    nc.compile()
    return nc


def run(inputs, ncores=NCORES, tiles_pc=TILES_PC, trace=False):
    from concourse.bass_utils import run_bass_kernel_spmd

    pre = preprocess(inputs, ncores=ncores, tiles_pc=tiles_pc)
    nc = build_nc(pre["chl"], pre["chh"], ncores=ncores, tiles_pc=tiles_pc,
                  nf=pre["nf"])
    consts = pre["consts"]
    in_maps = []
    for c in range(ncores):
        m = dict(
            xT=consts["xT"], xT_own=pre["xT_own"][c], rhs1=consts["rhs1"],
            rhs2=consts["rhs2"], b1_bc=consts["b1_bc"], b2_bc=consts["b2_bc"],
            J=consts["J"], eidx=pre["eidx"][c], edl=pre["edl"][c])
        in_maps.append(m)
    res = run_bass_kernel_spmd(nc, in_maps, core_ids=list(range(ncores)),
                               trace=trace)
    y = np.concatenate([res.results[c]["y"] for c in range(ncores)], axis=0)
    n_nodes = np.asarray(inputs["x"]).shape[0]
    return y[:n_nodes].astype(np.float32), res


def kernel(**inputs):
    y, _ = run(inputs)
    return y

